# revision 10
# baseline (speedup 1.0000x reference)
"""CoupledClustersLossV2 Trainium2 kernel (v4 — wide-DMA, pipelined drain).

Full inputs in, full output out. Shards embeddings [16384, 2048] f32
across 8 NeuronCores along the class axis (each core: 32 classes = 2048
rows), computes per-class losses on-core, means on the host.

Layout: each SBUF partition holds R consecutive HBM rows, so DMA packets
are R*8KiB contiguous HBM reads (the 16 DMA engines are packet-rate
bound: 8K->17, 16K->~23, 32K->26 GB/s each). R varies per supertile
(R_SEQ): big tiles for bulk bandwidth, small trailing tiles so the
load->matmul->sub->square chain of the LAST tile is short.

Per-core pipeline, per supertile [128 partitions, R rows, D=2048]:
  - anchor[m, d] = mean of the 32 pos rows of m's class via R
    PSUM-accumulated matmuls with constant A_R[p, m] = 1/32 * [p pos
    partition of m's class]; anchor lives as two [128, 1024] PSUM
    halves so downstream subs start after half the matmuls.
  - diff_r = x_r - anchor: VectorE, chunked [128, 1024].
  - d2[:, t] = sum_d diff_r^2: ScalarE Square+accum (full [128, 2048]).
  - d2 cols stream to a DRAM scratch (row order), and the class-major
    readback happens per supertile too, hiding DMA completion latency.
Tail: sqrt, per-class min / hinge / sum — all free-dim ops on [32, 64];
losses [32] DMA'd out. Host means 8x32 values.
"""

import sys

import numpy as np

for _p in ("/opt/trn_rl_repo",):
    if _p not in sys.path:
        sys.path.append(_p)

import concourse.bacc as bacc
import concourse.mybir as mybir
from concourse import tile
from concourse.bass_utils import run_bass_kernel_spmd

N_CORES = 8
D = 2048
S = 32                  # samples per class per polarity
ROWS_PER_CLASS = 2 * S  # 64: 32 pos then 32 neg
C_PER_CORE = 32         # classes per core (256 / 8)
ROWS_PER_CORE = C_PER_CORE * ROWS_PER_CLASS  # 2048

R_SEQ = (4, 4, 4, 2, 1, 1)  # rows-per-partition per supertile; sum = 16
MM_CHUNK = 512              # matmul free-dim (1 PSUM bank)
HALF = 1024                 # sub/anchor-half chunk
DMA_MODE = "hwdge"          # input loads: sync HWDGE only
TRACE = False

F32 = mybir.dt.float32
F32R = mybir.dt.float32r
AF = mybir.ActivationFunctionType
ALU = mybir.AluOpType

_CACHE = {}
LAST_RESULTS = None


def _a_matrix(r: int) -> np.ndarray:
    """A_r[p, m] = 1/32 if p is a pos partition of m's class (R=r layout)."""
    ppc = ROWS_PER_CLASS // r
    a = np.zeros((128, 128), dtype=np.float32)
    for m in range(128):
        c = m // ppc
        for pj in range(ppc // 2):
            a[c * ppc + pj, m] = 1.0 / S
    return a


def _inline_tensor(nc, data: np.ndarray, name: str, dtype):
    """nc.inline_tensor with an explicit BIR dtype (float32r over float32
    bytes — same width, so the embedded .npy payload stays valid)."""
    import base64
    import io

    import concourse.bass as bass

    data = np.ascontiguousarray(data)
    assert mybir.dt.size(dtype) == data.dtype.itemsize
    mls = nc._tensor(name, list(data.shape), dtype, kind="Const", type="DRAM")
    buf = io.BytesIO()
    np.save(buf, data, allow_pickle=False)
    mls.file = f"{name}.npy"
    mls.ant_data = base64.standard_b64encode(buf.getvalue()).decode()
    return bass.DRamTensorHandle(name, list(data.shape), dtype)


def _build(margin: float):
    assert sum(R_SEQ) * 128 == ROWS_PER_CORE
    nc = bacc.Bacc("TRN2", target_bir_lowering=False, debug=False)
    emb = nc.dram_tensor("emb", [ROWS_PER_CORE, D], F32R, kind="ExternalInput")
    out = nc.dram_tensor("losses", [C_PER_CORE], F32, kind="ExternalOutput")
    scratch = nc.dram_tensor("d2scratch", [ROWS_PER_CORE], F32, kind="Internal")

    a_consts = {
        r: _inline_tensor(nc, _a_matrix(r), f"amat{r}", F32R)
        for r in sorted(set(R_SEQ))
    }

    with tile.TileContext(nc) as tc:
        with (
            tc.tile_pool(name="consts", bufs=1) as cpool,
            tc.tile_pool(name="stats", bufs=1) as spool,
            tc.tile_pool(name="inp", bufs=4) as ipool,
            tc.tile_pool(name="dif", bufs=4) as dpool,
            tc.tile_pool(name="tail", bufs=1) as tpool,
        ):
            a_sb = {}
            for r, hnd in a_consts.items():
                a_sb[r] = cpool.tile(
                    [128, 128], F32R, tag=f"amat{r}", name=f"amat{r}_sb"
                )
                nc.gpsimd.dma_start(out=a_sb[r][:], in_=hnd[:, :])
            ncol = sum(R_SEQ)
            d2 = spool.tile([128, ncol], F32)
            d2c = tpool.tile([C_PER_CORE, ROWS_PER_CLASS], F32)

            emb_flat = emb[:, :].rearrange("n d -> (n d)")

            with tc.tile_pool(name="panc", bufs=4, space="PSUM") as ppool:
                base = 0  # row offset of current supertile
                col = 0   # d2 column offset
                for s_, R in enumerate(R_SEQ):
                    # partition p <- rows base + R*p .. base + R*p + R-1
                    src = emb_flat[D * base : D * (base + 128 * R)].rearrange(
                        "(p q) -> p q", p=128
                    )
                    x = ipool.tile([128, R * D], F32R, tag="x")
                    eng = nc.gpsimd if DMA_MODE == "swdge" else nc.sync
                    if DMA_MODE == "alt":
                        eng = nc.sync if s_ % 2 == 0 else nc.gpsimd
                    eng.dma_start(out=x[:, : R * D], in_=src)

                    halves = []
                    for h in range(D // HALF):
                        anc = ppool.tile([128, HALF], F32, tag="anc", name="anc")
                        halves.append(anc)
                        for c in range(HALF // MM_CHUNK):
                            lo = HALF * h + MM_CHUNK * c
                            hi = lo + MM_CHUNK
                            for r in range(R):
                                nc.tensor.matmul(
                                    anc[:, MM_CHUNK * c : MM_CHUNK * (c + 1)],
                                    a_sb[R][:],
                                    x[:, r * D + lo : r * D + hi],
                                    start=(r == 0),
                                    stop=(r == R - 1),
                                )
                    for r in range(R):
                        diff = dpool.tile([128, D], F32, tag="diff", name="diff")
                        for h in range(D // HALF):
                            xrh = x[
                                :, r * D + HALF * h : r * D + HALF * (h + 1)
                            ].bitcast(F32)
                            nc.vector.tensor_tensor(
                                diff[:, HALF * h : HALF * (h + 1)],
                                xrh,
                                halves[h][:],
                                op=ALU.subtract,
                            )
                        nc.scalar.activation(
                            diff[:],
                            diff[:],
                            AF.Square,
                            accum_out=d2[:, col + r : col + r + 1],
                        )
                    # stream this supertile's d2 cols out and read them back
                    # class-major; per-supertile so the DMA completion
                    # latency hides under the next supertile's compute.
                    nc.sync.dma_start(
                        out=scratch[base : base + 128 * R].rearrange(
                            "(p r) -> p r", p=128
                        ),
                        in_=d2[:, col : col + R],
                    )
                    c0 = base // ROWS_PER_CLASS
                    c1 = (base + 128 * R) // ROWS_PER_CLASS
                    nc.sync.dma_start(
                        out=d2c[c0:c1, :],
                        in_=scratch[base : base + 128 * R].rearrange(
                            "(c i) -> c i", c=c1 - c0
                        ),
                    )
                    base += 128 * R
                    col += R

            # ACT table preload: pull Sqrt in while the last squares
            # retire; input depends on a late d2 column so the scheduler
            # keeps it near the end of the Square stream.
            warm = tpool.tile([1, 1], F32)
            nc.scalar.activation(warm[:], d2[0:1, ncol - 1 : ncol], AF.Sqrt)

            dist = tpool.tile([C_PER_CORE, ROWS_PER_CLASS], F32)
            nc.scalar.activation(dist[:], d2c[:], AF.Sqrt)
            an = tpool.tile([C_PER_CORE, 1], F32)
            nc.vector.tensor_reduce(
                an[:], dist[:, S:], axis=mybir.AxisListType.X, op=ALU.min
            )
            anm = tpool.tile([C_PER_CORE, 1], F32)
            nc.vector.tensor_scalar(
                anm[:], an[:], float(margin), None, op0=ALU.subtract
            )
            hinge = tpool.tile([C_PER_CORE, S], F32)
            nc.vector.tensor_scalar(
                hinge[:], dist[:, :S], anm[:], 0.0, op0=ALU.subtract, op1=ALU.max
            )
            hsq = tpool.tile([C_PER_CORE, S], F32)
            nc.vector.tensor_tensor(hsq[:], hinge[:], hinge[:], op=ALU.mult)
            losses = tpool.tile([C_PER_CORE, 1], F32)
            nc.vector.tensor_reduce(
                losses[:], hsq[:], axis=mybir.AxisListType.X, op=ALU.add
            )
            nc.sync.dma_start(out=out[:], in_=losses[:, 0])

    nc.compile()
    return nc


def kernel(embeddings, target=None, margin=0.3, n_classes=256, n_samples=32, **_):
    global LAST_RESULTS
    emb = np.ascontiguousarray(np.asarray(embeddings, dtype=np.float32))
    assert emb.shape == (16384, 2048), emb.shape
    assert int(n_classes) == 256 and int(n_samples) == 32

    key = (float(margin), R_SEQ, DMA_MODE)
    nc = _CACHE.get(key)
    if nc is None:
        nc = _CACHE[key] = _build(float(margin))

    shards = emb.reshape(N_CORES, ROWS_PER_CORE, D)
    in_maps = [{"emb": shards[c]} for c in range(N_CORES)]
    res = run_bass_kernel_spmd(
        nc, in_maps, core_ids=list(range(N_CORES)), trace=TRACE
    )
    LAST_RESULTS = res
    per_class = np.concatenate([r["losses"].reshape(-1) for r in res.results])
    return np.float32(per_class.mean())


# revision 12
# speedup vs baseline: 1.0750x; 1.0750x over previous
"""CoupledClustersLossV2 Trainium2 kernel (v4 — wide-DMA, pipelined drain).

Full inputs in, full output out. Shards embeddings [16384, 2048] f32
across 8 NeuronCores along the class axis (each core: 32 classes = 2048
rows), computes per-class losses on-core, means on the host.

Layout: each SBUF partition holds R consecutive HBM rows, so DMA packets
are R*8KiB contiguous HBM reads (the 16 DMA engines are packet-rate
bound: 8K->17, 16K->~23, 32K->26 GB/s each). R varies per supertile
(R_SEQ): big tiles for bulk bandwidth, small trailing tiles so the
load->matmul->sub->square chain of the LAST tile is short.

Per-core pipeline, per supertile [128 partitions, R rows, D=2048]:
  - anchor[m, d] = mean of the 32 pos rows of m's class via R
    PSUM-accumulated matmuls with constant A_R[p, m] = 1/32 * [p pos
    partition of m's class]; anchor lives as two [128, 1024] PSUM
    halves so downstream subs start after half the matmuls.
  - diff_r = x_r - anchor: VectorE, chunked [128, 1024].
  - d2[:, t] = sum_d diff_r^2: ScalarE Square+accum (full [128, 2048]).
  - d2 cols stream to a DRAM scratch (row order), and the class-major
    readback happens per supertile too, hiding DMA completion latency.
Tail: sqrt, per-class min / hinge / sum — all free-dim ops on [32, 64];
losses [32] DMA'd out. Host means 8x32 values.
"""

import sys

import numpy as np

for _p in ("/opt/trn_rl_repo",):
    if _p not in sys.path:
        sys.path.append(_p)

import concourse.bacc as bacc
import concourse.mybir as mybir
from concourse import tile
from concourse.bass_utils import run_bass_kernel_spmd

N_CORES = 8
D = 2048
S = 32                  # samples per class per polarity
ROWS_PER_CLASS = 2 * S  # 64: 32 pos then 32 neg
C_PER_CORE = 32         # classes per core (256 / 8)
ROWS_PER_CORE = C_PER_CORE * ROWS_PER_CLASS  # 2048

R_SEQ = (4, 4, 4, 2, 1, 1)  # rows-per-partition per supertile; sum = 16
MM_CHUNK = 512              # matmul free-dim (1 PSUM bank)
HALF = 1024                 # sub/anchor-half chunk
DMA_MODE = "hwdge"          # input loads: sync HWDGE only
TRACE = False

F32 = mybir.dt.float32
F32R = mybir.dt.float32r
AF = mybir.ActivationFunctionType
ALU = mybir.AluOpType

_CACHE = {}
LAST_RESULTS = None


def _a_matrix(r: int) -> np.ndarray:
    """A_r[p, m] = 1/32 if p is a pos partition of m's class (R=r layout)."""
    ppc = ROWS_PER_CLASS // r
    a = np.zeros((128, 128), dtype=np.float32)
    for m in range(128):
        c = m // ppc
        for pj in range(ppc // 2):
            a[c * ppc + pj, m] = 1.0 / S
    return a


def _inline_tensor(nc, data: np.ndarray, name: str, dtype):
    """nc.inline_tensor with an explicit BIR dtype (float32r over float32
    bytes — same width, so the embedded .npy payload stays valid)."""
    import base64
    import io

    import concourse.bass as bass

    data = np.ascontiguousarray(data)
    assert mybir.dt.size(dtype) == data.dtype.itemsize
    mls = nc._tensor(name, list(data.shape), dtype, kind="Const", type="DRAM")
    buf = io.BytesIO()
    np.save(buf, data, allow_pickle=False)
    mls.file = f"{name}.npy"
    mls.ant_data = base64.standard_b64encode(buf.getvalue()).decode()
    return bass.DRamTensorHandle(name, list(data.shape), dtype)


def _build(margin: float):
    assert sum(R_SEQ) * 128 == ROWS_PER_CORE
    nc = bacc.Bacc("TRN2", target_bir_lowering=False, debug=False)
    emb = nc.dram_tensor("emb", [ROWS_PER_CORE, D], F32R, kind="ExternalInput")
    out = nc.dram_tensor("losses", [C_PER_CORE], F32, kind="ExternalOutput")
    scratch = nc.dram_tensor("d2scratch", [ROWS_PER_CORE], F32, kind="Internal")

    a_consts = {
        r: _inline_tensor(nc, _a_matrix(r), f"amat{r}", F32R)
        for r in sorted(set(R_SEQ))
    }

    with tile.TileContext(nc) as tc:
        with (
            tc.tile_pool(name="consts", bufs=1) as cpool,
            tc.tile_pool(name="stats", bufs=1) as spool,
            tc.tile_pool(name="inp", bufs=4) as ipool,
            tc.tile_pool(name="dif", bufs=4) as dpool,
            tc.tile_pool(name="tail", bufs=1) as tpool,
        ):
            a_sb = {}
            for r, hnd in a_consts.items():
                a_sb[r] = cpool.tile(
                    [128, 128], F32R, tag=f"amat{r}", name=f"amat{r}_sb"
                )
                nc.gpsimd.dma_start(out=a_sb[r][:], in_=hnd[:, :])
            ncol = sum(R_SEQ)
            d2 = spool.tile([128, ncol], F32)
            d2c = tpool.tile([C_PER_CORE, ROWS_PER_CLASS], F32)

            emb_flat = emb[:, :].rearrange("n d -> (n d)")

            with tc.tile_pool(name="panc", bufs=4, space="PSUM") as ppool:
                base = 0  # row offset of current supertile
                col = 0   # d2 column offset
                for s_, R in enumerate(R_SEQ):
                    # partition p <- rows base + R*p .. base + R*p + R-1
                    src = emb_flat[D * base : D * (base + 128 * R)].rearrange(
                        "(p q) -> p q", p=128
                    )
                    x = ipool.tile([128, R * D], F32R, tag="x")
                    eng = nc.gpsimd if DMA_MODE == "swdge" else nc.sync
                    if DMA_MODE == "alt":
                        eng = nc.sync if s_ % 2 == 0 else nc.gpsimd
                    eng.dma_start(out=x[:, : R * D], in_=src)

                    halves = []
                    for h in range(D // HALF):
                        anc = ppool.tile([128, HALF], F32, tag="anc", name="anc")
                        halves.append(anc)
                        for c in range(HALF // MM_CHUNK):
                            lo = HALF * h + MM_CHUNK * c
                            hi = lo + MM_CHUNK
                            for r in range(R):
                                nc.tensor.matmul(
                                    anc[:, MM_CHUNK * c : MM_CHUNK * (c + 1)],
                                    a_sb[R][:],
                                    x[:, r * D + lo : r * D + hi],
                                    start=(r == 0),
                                    stop=(r == R - 1),
                                )
                    for r in range(R):
                        diff = dpool.tile([128, D], F32, tag="diff", name="diff")
                        for h in range(D // HALF):
                            xrh = x[
                                :, r * D + HALF * h : r * D + HALF * (h + 1)
                            ].bitcast(F32)
                            nc.vector.tensor_tensor(
                                diff[:, HALF * h : HALF * (h + 1)],
                                xrh,
                                halves[h][:],
                                op=ALU.subtract,
                            )
                        nc.scalar.activation(
                            diff[:],
                            diff[:],
                            AF.Square,
                            accum_out=d2[:, col + r : col + r + 1],
                        )
                    # stream this supertile's d2 cols out and read them back
                    # class-major; per-supertile so the DMA completion
                    # latency hides under the next supertile's compute.
                    # On the gpsimd (SWDGE) queue: the sync queue must hold
                    # ONLY the input loads — a DMA behind them would block
                    # later loads (head-of-line), and waiting on squares
                    # here would stall the whole load stream.
                    nc.gpsimd.dma_start(
                        out=scratch[base : base + 128 * R].rearrange(
                            "(p r) -> p r", p=128
                        ),
                        in_=d2[:, col : col + R],
                    )
                    c0 = base // ROWS_PER_CLASS
                    c1 = (base + 128 * R) // ROWS_PER_CLASS
                    nc.gpsimd.dma_start(
                        out=d2c[c0:c1, :],
                        in_=scratch[base : base + 128 * R].rearrange(
                            "(c i) -> c i", c=c1 - c0
                        ),
                    )
                    base += 128 * R
                    col += R

            # ACT table preload: pull Sqrt in while the last squares
            # retire; input depends on a late d2 column so the scheduler
            # keeps it near the end of the Square stream.
            warm = tpool.tile([1, 1], F32)
            nc.scalar.activation(warm[:], d2[0:1, ncol - 1 : ncol], AF.Sqrt)

            dist = tpool.tile([C_PER_CORE, ROWS_PER_CLASS], F32)
            nc.scalar.activation(dist[:], d2c[:], AF.Sqrt)
            an = tpool.tile([C_PER_CORE, 1], F32)
            nc.vector.tensor_reduce(
                an[:], dist[:, S:], axis=mybir.AxisListType.X, op=ALU.min
            )
            anm = tpool.tile([C_PER_CORE, 1], F32)
            nc.vector.tensor_scalar(
                anm[:], an[:], float(margin), None, op0=ALU.subtract
            )
            hinge = tpool.tile([C_PER_CORE, S], F32)
            nc.vector.tensor_scalar(
                hinge[:], dist[:, :S], anm[:], 0.0, op0=ALU.subtract, op1=ALU.max
            )
            hsq = tpool.tile([C_PER_CORE, S], F32)
            nc.vector.tensor_tensor(hsq[:], hinge[:], hinge[:], op=ALU.mult)
            losses = tpool.tile([C_PER_CORE, 1], F32)
            nc.vector.tensor_reduce(
                losses[:], hsq[:], axis=mybir.AxisListType.X, op=ALU.add
            )
            nc.gpsimd.dma_start(out=out[:], in_=losses[:, 0])

    nc.compile()
    return nc


def kernel(embeddings, target=None, margin=0.3, n_classes=256, n_samples=32, **_):
    global LAST_RESULTS
    emb = np.ascontiguousarray(np.asarray(embeddings, dtype=np.float32))
    assert emb.shape == (16384, 2048), emb.shape
    assert int(n_classes) == 256 and int(n_samples) == 32

    key = (float(margin), R_SEQ, DMA_MODE)
    nc = _CACHE.get(key)
    if nc is None:
        nc = _CACHE[key] = _build(float(margin))

    shards = emb.reshape(N_CORES, ROWS_PER_CORE, D)
    in_maps = [{"emb": shards[c]} for c in range(N_CORES)]
    res = run_bass_kernel_spmd(
        nc, in_maps, core_ids=list(range(N_CORES)), trace=TRACE
    )
    LAST_RESULTS = res
    per_class = np.concatenate([r["losses"].reshape(-1) for r in res.results])
    return np.float32(per_class.mean())
